# revision 7
# baseline (speedup 1.0000x reference)
"""E2: pure-fp16 decoder kernel with super-granular swizzled DMA.

Host pre-swizzles xT16/wT16 into [NSUP, 128, SUP, free] so each super
loads with 2 big DMAs per operand (split across the two HWDGE queues)
instead of 32 chunk DMAs -> ~4 semaphore waits per super instead of 32.
Tests whether fine-grained DMA sync causes the diffuse per-matmul
inflation (131 vs 109 ns / 259 vs 216 ns).
"""

import os
import sys

if "/opt/trn_rl_repo" not in sys.path:
    sys.path.insert(0, "/opt/trn_rl_repo")

import numpy as np

N_TOK = 8192
D_IN = 32768
D_OUT = 768
N_CORES = 8
N_SHARD = N_TOK // N_CORES
P = 128
SUP = 16                      # fp16 chunks (128 rows) per super
NSUP = D_IN // (SUP * P)      # 16 supers
XS_BUFS = int(os.environ.get("KERNEL_XS_BUFS", "2"))
WS_BUFS = int(os.environ.get("KERNEL_WS_BUFS", "2"))
PS_BUFS = int(os.environ.get("KERNEL_PS_BUFS", "3"))

LAST_RESULTS = None


def _build_bass():
    import concourse.mybir as mybir
    import concourse.tile as tile
    from concourse import bacc

    fp16 = mybir.dt.float16
    f32 = mybir.dt.float32
    NCH = N_SHARD // P
    H = SUP // 2

    nc = bacc.Bacc(None, target_bir_lowering=False)
    xS = nc.dram_tensor("xS", [NSUP, 2, P, H, N_SHARD], fp16,
                        kind="ExternalInput")
    wS = nc.dram_tensor("wS", [NSUP, 2, P, H, D_OUT], fp16,
                        kind="ExternalInput")
    out = nc.dram_tensor("out", [N_SHARD, D_OUT], f32,
                         kind="ExternalOutput")

    with tile.TileContext(nc) as tc:
        with (
            tc.tile_pool(name="xs", bufs=XS_BUFS) as xpool,
            tc.tile_pool(name="ws", bufs=WS_BUFS) as wpool,
            tc.tile_pool(name="c", bufs=1) as cpool,
            tc.tile_pool(name="psum", bufs=PS_BUFS, space="PSUM") as ppool,
        ):
            cts = [
                cpool.tile([P, D_OUT], f32, name=f"c{i}") for i in range(NCH)
            ]
            for s in range(NSUP):
                # two half-super tiles per operand, one per DMA queue
                xt = [None, None]
                wt = [None, None]
                for h in range(2):
                    xt[h] = xpool.tile([P, H, N_SHARD], fp16, name=f"xt{h}")
                    wt[h] = wpool.tile([P, H, D_OUT], fp16, name=f"wt{h}")
                    # x halves on sync queue, w halves on scalar queue
                    nc.sync.dma_start(xt[h][:], xS[s, h])
                    nc.scalar.dma_start(wt[h][:], wS[s, h])
                for nch in range(NCH):
                    ps = ppool.tile([P, D_OUT], f32, name="ps")
                    for j in range(SUP):
                        h, jj = divmod(j, H)
                        lhsT = xt[h][:, jj, nch * P:(nch + 1) * P]
                        st = j == 0
                        sp = j == SUP - 1
                        nc.tensor.matmul(ps[:, 0:512], lhsT,
                                         wt[h][:, jj, 0:512],
                                         start=st, stop=sp)
                        nc.tensor.matmul(ps[:, 512:D_OUT], lhsT,
                                         wt[h][:, jj, 512:D_OUT],
                                         start=st, stop=sp)
                    if s == 0:
                        nc.vector.tensor_copy(cts[nch][:], ps[:])
                    else:
                        nc.vector.tensor_add(cts[nch][:], cts[nch][:], ps[:])
            for nch in range(NCH):
                nc.sync.dma_start(out[nch * P:(nch + 1) * P, :], cts[nch][:])

    nc.compile()
    return nc


def _prep_inputs(x, W):
    H = SUP // 2
    wT = W.T.astype(np.float16)                     # [D_IN, D_OUT]
    wSv = np.ascontiguousarray(
        wT.reshape(NSUP, 2, H, P, D_OUT).transpose(0, 1, 3, 2, 4))
    in_maps = []
    for c in range(N_CORES):
        xsT = x[c * N_SHARD:(c + 1) * N_SHARD].T.astype(np.float16)
        xSv = np.ascontiguousarray(
            xsT.reshape(NSUP, 2, H, P, N_SHARD).transpose(0, 1, 3, 2, 4))
        in_maps.append({"xS": xSv, "wS": wSv})
    return in_maps


def kernel(x: np.ndarray, W: np.ndarray, b_pre: np.ndarray) -> np.ndarray:
    global LAST_RESULTS
    from concourse.bass_utils import run_bass_kernel_spmd

    x = np.asarray(x, dtype=np.float32)
    W = np.asarray(W, dtype=np.float32)
    b_pre = np.asarray(b_pre, dtype=np.float32)
    if b_pre.any():
        x = x - b_pre[None, :]

    nc = _build_bass()
    in_maps = _prep_inputs(x, W)
    last_err = None
    for attempt in range(3):
        try:
            LAST_RESULTS = run_bass_kernel_spmd(
                nc, in_maps, core_ids=list(range(N_CORES)),
                tmpdir=os.environ.get("KERNEL_TRACE_DIR") or None,
            )
            break
        except Exception as e:
            last_err = e
            import time

            time.sleep(10)
    else:
        raise last_err
    return np.concatenate(
        [LAST_RESULTS.results[c]["out"] for c in range(N_CORES)], axis=0
    )
